# revision 72
# baseline (speedup 1.0000x reference)
"""Trainium2 Bass kernel for multi-head attention (B=4, L=2048, D=1024, H=16).

Sharding: 8 cores = 4 batches x 2 head-groups (8 heads each).
v2: head-pair-outer schedule with projections interleaved as fillers so the
ACT engine (exp, the ~254us floor) starts at ~10us instead of ~100us, and a
transposed PV matmul (out[qpos, feat], N=65) that halves PV's PE cost; the
softmax denominator rides along as a ones-column (per-partition scalar after
transposition), and attn output reaches the feature-major layout the output
projection needs via xbar DMA transposes on the otherwise-idle DMA engines.

All matmul operands bf16 (fp32 PSUM accumulation); output fp32.
"""
import sys

sys.path.insert(0, "/opt/trn_rl_repo")
import numpy as np
import concourse.bass as bass
import concourse.bacc as bacc
import concourse.mybir as mybir
from concourse.tile import TileContext
from concourse.bass_utils import run_bass_kernel_spmd

L = 2048          # sequence length
D = 1024          # model dim
HD = 64           # head dim
F_V = 512         # v features per core
KT = L // 128     # 16 k position tiles
QH = 8            # q half-chunks of 256
DT = mybir.dt.bfloat16
F32 = mybir.dt.float32
SCALE = HD ** -0.5
AF = mybir.ActivationFunctionType
GROUPS = [(0, 1, 2, 3), (4, 5, 6, 7), (8, 9, 10, 11), (12, 13, 14, 15)]


def build_nc():
    nc = bacc.Bacc("TRN2", target_bir_lowering=False, debug=False, num_devices=8)
    xT = nc.dram_tensor("xT", [D, L], DT, kind="ExternalInput")
    wqkT = nc.dram_tensor("wqkT", [8, 128, 8, 128], DT, kind="ExternalInput")
    wvT = nc.dram_tensor("wvT", [D, F_V], DT, kind="ExternalInput")
    bqk = nc.dram_tensor("bqk", [128, 8], F32, kind="ExternalInput")
    bv = nc.dram_tensor("bv", [1, F_V], DT, kind="ExternalInput")
    woT = nc.dram_tensor("woT", [F_V, D], DT, kind="ExternalInput")
    bout = nc.dram_tensor("bout", [1, D], DT, kind="ExternalInput")
    cosT = nc.dram_tensor("cosT", [128, L], DT, kind="ExternalInput")
    sinT = nc.dram_tensor("sinT", [128, L], DT, kind="ExternalInput")
    out = nc.dram_tensor("out", [L, D], F32, kind="ExternalOutput")

    with TileContext(nc) as tc:
        with (
            tc.tile_pool(name="const", bufs=1) as cp,
            tc.tile_pool(name="wstream", bufs=2) as wsp,
            tc.tile_pool(name="rope", bufs=2) as rp,
            tc.tile_pool(name="exps", bufs=1) as ep,
            tc.tile_pool(name="small", bufs=4) as sp,
            tc.tile_pool(name="psum", bufs=1, space="PSUM") as pp,
        ):
            dma = nc.sync
            dma2 = nc.gpsimd

            # ---- resident inputs ----
            # wqk for k of hp0 first (header need), then x chunks on both
            # queues; later fcs stream in as fillers.
            wqk_tiles = {}

            def wqk_load(fc):
                wqk_tiles[fc] = wsp.tile([128, 8, 128], DT, tag="wqk",
                                         name=f"wqk{fc}", bufs=3)
                dma.dma_start(out=wqk_tiles[fc][:], in_=wqkT[fc])

            # load order tuned for the header critical path: x half-L
            # slices split across both DMA queues (first QKPs need only the
            # first halves), rope tables tiny-first on the Pool queue, x
            # half1 and bulk weights emitted mid-header after the first
            # rope copies
            wqk_load(4)
            xT_sb = cp.tile([128, 8, L], DT)        # x.T  [d-chunk, chunk, l]
            wvT_sb = cp.tile([128, 8, F_V], DT)
            cos_sb = cp.tile([128, L], DT)
            dma2.dma_start(out=cos_sb[:], in_=cosT[:])
            sin_sb = cp.tile([128, L], DT)
            dma2.dma_start(out=sin_sb[:], in_=sinT[:])
            bqk_sb = cp.tile([128, 8], F32)
            dma2.dma_start(out=bqk_sb[:], in_=bqk[:])
            bv_sb = cp.tile([1, F_V], DT)
            dma2.dma_start(out=bv_sb[:], in_=bv[:])

            def x_load(h):
                # split across both DMA queues (evens sync, odds swdge) so
                # the header's contraction chunks arrive ~2x faster; half1
                # is emitted mid-header so rope copies jump ahead of it
                hs = slice(h * 1024, (h + 1) * 1024)
                for c in range(8):
                    (dma, dma2)[c % 2].dma_start(out=xT_sb[:, c, hs],
                                                 in_=xT[c * 128:(c + 1) * 128, hs])

            x_load(0)
            wqk_load(0)
            bout_sb = cp.tile([1, D], DT)
            woT_sb = cp.tile([128, 4, D], DT)
            bv_bc = cp.tile([128, F_V], DT)
            bout_bc = cp.tile([128, D], DT)

            def wv_load():
                for c in range(4):
                    dma.dma_start(out=wvT_sb[:, c, :],
                                  in_=wvT[c * 128:(c + 1) * 128, :])
                for c in range(4, 8):
                    dma2.dma_start(out=wvT_sb[:, c, :],
                                   in_=wvT[c * 128:(c + 1) * 128, :])
                nc.gpsimd.partition_broadcast(bv_bc[:], bv_sb[:])

            def const_tail():
                dma.dma_start(out=bout_sb[:], in_=bout[:])
                for c in range(4):
                    dma.dma_start(out=woT_sb[:, c, :],
                                  in_=woT[c * 128:(c + 1) * 128, :])
                nc.gpsimd.partition_broadcast(bout_bc[:], bout_sb[:])

            # q (c 0-3) / k (c 4-7), feature-major; separate tiles per c so
            # late rope writes never alias earlier scores' reads
            qk_c = [cp.tile([128, L], DT, name=f"qk{c}") for c in range(8)]
            # V: [kpos, kt, hp, vA(64) 1A vB(64) 1B]; ones col = softmax denom
            V_sb = cp.tile([128, KT, 4, 130], DT)
            ones_cols = V_sb.rearrange("p t h (b c) -> p t h b c", c=65)[:, :, :, :, 64]
            nc.vector.memset(ones_cols, 1.0)
            # attn output, feature-major, per q-chunk; alive all run
            cT4 = [cp.tile([128, 4, 512], DT, name=f"cT{qc}") for qc in range(4)]

            # ---- work units ----
            def qk_proj(fc, nt, ptag="pp"):
                if fc in wqk_tiles:
                    wqk_t = wqk_tiles[fc]
                else:
                    wqk_load(fc)
                    wqk_t = wqk_tiles[fc]
                qps = pp.tile([128, 512], F32, tag=ptag, name=f"qps{fc}_{nt}",
                              bufs=2 if ptag == "pp" else 1)
                for kc in range(8):
                    nc.tensor.matmul(qps[:], lhsT=wqk_t[:, kc, :],
                                     rhs=xT_sb[:, kc, nt * 512:(nt + 1) * 512],
                                     start=(kc == 0), stop=(kc == 7))
                nc.vector.tensor_scalar_add(qk_c[fc][:, nt * 512:(nt + 1) * 512],
                                            qps[:], bqk_sb[:, fc:fc + 1])

            def rope(c, nt=None):
                # copies split across the two DMA queues; arithmetic on DVE
                # (Pool's tensor ops are 3x slower, no 2x bf16 mode).
                # nt=None ropes the whole row (filler path, 4x fewer DMAs)
                if nt is None:
                    s = slice(0, L)
                    w, rtag, ttag = L, "rotW", "rtmpW"
                else:
                    s = slice(nt * 512, (nt + 1) * 512)
                    w, rtag, ttag = 512, "rot", "rtmp"
                rot = rp.tile([128, w], DT, tag=rtag, name=f"rot{c}_{nt}")
                for h2 in range(2):
                    p = 64 * h2
                    (dma, dma2)[h2].dma_start(out=rot[p:p + 32, :],
                                              in_=qk_c[c][p + 32:p + 64, s])
                    (dma2, dma)[h2].dma_start(out=rot[p + 32:p + 64, :],
                                              in_=qk_c[c][p:p + 32, s])
                tmp = rp.tile([128, w], DT, tag=ttag, name=f"rtmp{c}_{nt}")
                nc.vector.tensor_mul(tmp[:], qk_c[c][:, s], cos_sb[:, s])
                nc.vector.tensor_mul(rot[:], rot[:], sin_sb[:, s])
                nc.vector.tensor_add(qk_c[c][:, s], tmp[:], rot[:])

            def v_proj(lt, hp):
                vps = pp.tile([128, 128], F32, tag="pp", bufs=2,
                              name=f"vps{lt}_{hp}")
                for kc in range(8):
                    nc.tensor.matmul(vps[:],
                                     lhsT=xT_sb[:, kc, lt * 128:(lt + 1) * 128],
                                     rhs=wvT_sb[:, kc, hp * 128:(hp + 1) * 128],
                                     start=(kc == 0), stop=(kc == 7))
                for h2 in range(2):
                    nc.vector.tensor_add(
                        V_sb[:, lt, hp, h2 * 65:h2 * 65 + 64],
                        vps[:, h2 * 64:(h2 + 1) * 64],
                        bv_bc[:, hp * 128 + h2 * 64:hp * 128 + (h2 + 1) * 64])

            def out_proj_half(qc, mq, dt_, osb):
                # one d-half per filler slot (0.85us fits the inter-group
                # PE budget); merged [128, 1024] store after the second half
                ops = pp.tile([128, 512], F32, tag="pp", bufs=2,
                              name=f"ops{qc}_{dt_}_{mq}")
                for cc in range(4):
                    nc.tensor.matmul(ops[:],
                                     lhsT=cT4[qc][:, cc, mq * 128:(mq + 1) * 128],
                                     rhs=woT_sb[:, cc, dt_ * 512:(dt_ + 1) * 512],
                                     start=(cc == 0), stop=(cc == 3))
                nc.vector.tensor_add(osb[:, dt_ * 512:(dt_ + 1) * 512], ops[:],
                                     bout_bc[:, dt_ * 512:(dt_ + 1) * 512])
                if dt_ == 1:
                    dma.dma_start(
                        out=out[qc * 512 + mq * 128: qc * 512 + (mq + 1) * 128, :],
                        in_=osb[:])

            # ---- attention iteration state (hp, qh): 256 q positions,
            # 2 heads; software-pipelined: S(i+1, g0) is emitted before
            # iteration i's pv runs so ACT never waits out the PV block
            def make_iter(hp, qh):
                qs = slice(qh * 256, (qh + 1) * 256)
                expA = ep.tile([128, KT, 256], DT, tag="expA", bufs=2,
                               name=f"expA{hp}_{qh}")
                expB = ep.tile([128, KT, 256], DT, tag="expB", bufs=2,
                               name=f"expB{hp}_{qh}")
                pv = pp.tile([128, 2, 130], F32, tag="pv", bufs=2,
                             name=f"pv{hp}_{qh}")

                def s_group(gi):
                    kts = GROUPS[gi]
                    spsA = pp.tile([128, 1024], F32, tag="sA", name=f"sA{hp}_{qh}_{gi}")
                    spsB = pp.tile([128, 1024], F32, tag="sB", name=f"sB{hp}_{qh}_{gi}")
                    for j, kt in enumerate(kts):
                        for p, sps in ((0, spsA), (64, spsB)):
                            nc.tensor.matmul(
                                sps[:, j * 256:(j + 1) * 256],
                                lhsT=qk_c[4 + hp][p:p + 64, kt * 128:(kt + 1) * 128],
                                rhs=qk_c[hp][p:p + 64, qs],
                                start=True, stop=True)
                    n = len(kts)
                    k0 = kts[0]
                    nc.scalar.activation(
                        expA[:, k0:k0 + n, :].rearrange("p a b -> p (a b)"),
                        spsA[:, 0:n * 256], AF.Exp, scale=SCALE)
                    nc.scalar.activation(
                        expB[:, k0:k0 + n, :].rearrange("p a b -> p (a b)"),
                        spsB[:, 0:n * 256], AF.Exp, scale=SCALE)

                def pv_run(m, h2):
                    # one full contiguous accumulation run: interleaving
                    # different accumulation groups within one PSUM bank
                    # corrupts results (other-bank matmuls between runs are
                    # fine)
                    expS = (expA, expB)[h2]
                    for kt in range(KT):
                        nc.tensor.matmul(
                            pv[:, m, h2 * 65:(h2 + 1) * 65],
                            lhsT=expS[:, kt, m * 128:(m + 1) * 128],
                            rhs=V_sb[:, kt, hp, h2 * 65:(h2 + 1) * 65],
                            start=(kt == 0), stop=(kt == KT - 1))

                def div_m(m):
                    # divide by denominator (ones-column), cast bf16, transpose
                    rr = sp.tile([128, 2], F32, tag="rr", bufs=4,
                                 name=f"rr{hp}_{qh}_{m}")
                    nc.vector.reciprocal(
                        rr[:], pv[:, m].rearrange("p (h c) -> p h c", c=65)[:, :, 64])
                    qc, half = divmod(qh, 2)
                    aq = sp.tile([128, 128], DT, tag="aq", bufs=4,
                                 name=f"aq{hp}_{qh}_{m}")
                    for h2 in range(2):
                        nc.vector.tensor_scalar_mul(
                            aq[:, h2 * 64:(h2 + 1) * 64],
                            pv[:, m, h2 * 65:h2 * 65 + 64],
                            rr[:, h2:h2 + 1])
                    off = half * 256 + m * 128
                    dma.dma_start_transpose(out=cT4[qc][:, hp, off:off + 128],
                                            in_=aq[:])

                def div_tr():
                    div_m(0)
                    div_m(1)
                return s_group, pv_run, div_tr, div_m

            def iteration(cur, nxt, fillers, extra=(), pending_div=None, cap=2):
                # cur = (s_group, pv_run, div_tr); cur's s_group(0) was
                # already emitted by the previous iteration
                s_group, pv_run = cur[0], cur[1]
                if pending_div is not None:
                    # previous iteration's divide+transpose, deferred so the
                    # DVE reciprocal never head-of-line-blocks behind a
                    # still-accumulating pv bank, and the SP-queue transpose
                    # never waits on a fresh ts_mul
                    pending_div()
                extra = list(extra)
                budget = [cap]

                def fill(n=1):
                    # extras are pinned to this iteration; rationed fillers
                    # spread across the phase so late iterations keep PE
                    # cover during exp waits
                    for _ in range(n):
                        if extra:
                            extra.pop(0)()
                        elif fillers and budget[0] > 0:
                            budget[0] -= 1
                            fillers.pop(0)()

                fill()
                s_group(1)
                fill()
                s_group(2)
                fill()
                s_group(3)
                fill()
                if nxt is not None:
                    nxt[0](0)      # pipelined S(i+1, g0)
                pv_run(0, 0)
                pv_run(0, 1)
                fill()
                if nxt is None:
                    # drain fast: m0's divide/transpose overlaps m1's pv
                    cur[3](0)
                pv_run(1, 0)
                pv_run(1, 1)
                fill()
                if nxt is None:
                    cur[3](1)
                    return None
                return cur[2]

            # ---- schedule ----
            # header: k(hp0) nt0-2, q(hp0) nt0 + ropes
            # ((4,3) rides as an it0 extra so S(g0) isn't pushed back)
            qk_proj(4, 0, "sA")
            rope(4, 0)
            qk_proj(0, 0, "sB")
            rope(0, 0)
            qk_proj(4, 1, "pp")
            rope(4, 1)
            wv_load()
            x_load(1)

            def qkp_unit(fc, nt, do_rope=True, whole_rope=False):
                def emit():
                    qk_proj(fc, nt)
                    if whole_rope:
                        rope(fc)        # whole-L rope after the last nt
                    elif do_rope:
                        rope(fc, nt)
                return emit

            def qkp_units(fc, nt, whole_rope=False):
                # two 0.85us halves of the 8-matmul contraction; the psum
                # accumulation group safely spans other-bank matmuls
                state = {}

                def emit0():
                    if fc not in wqk_tiles:
                        wqk_load(fc)
                    state["qps"] = pp.tile([128, 512], F32, tag="pp", bufs=2,
                                           name=f"qps{fc}_{nt}")
                    for kc in range(4):
                        nc.tensor.matmul(state["qps"][:],
                                         lhsT=wqk_tiles[fc][:, kc, :],
                                         rhs=xT_sb[:, kc, nt * 512:(nt + 1) * 512],
                                         start=(kc == 0), stop=False)

                def emit1():
                    for kc in range(4, 8):
                        nc.tensor.matmul(state["qps"][:],
                                         lhsT=wqk_tiles[fc][:, kc, :],
                                         rhs=xT_sb[:, kc, nt * 512:(nt + 1) * 512],
                                         start=False, stop=(kc == 7))
                    nc.vector.tensor_scalar_add(
                        qk_c[fc][:, nt * 512:(nt + 1) * 512],
                        state["qps"][:], bqk_sb[:, fc:fc + 1])
                    if whole_rope:
                        rope(fc)
                return [emit0, emit1]

            def load_unit(*fcs):
                def emit():
                    for fc in fcs:
                        if fc not in wqk_tiles:
                            wqk_load(fc)
                return emit

            def vp_unit(hp, lts):
                def emit():
                    for lt in lts:
                        v_proj(lt, hp)
                return emit

            def op_units(qc, mq):
                state = {}

                def emit0():
                    state["osb"] = sp.tile([128, D], F32, tag="osb", bufs=2,
                                           name=f"osb{qc}_{mq}")
                    out_proj_half(qc, mq, 0, state["osb"])

                def emit1():
                    out_proj_half(qc, mq, 1, state["osb"])
                return [emit0, emit1]

            def phase_fillers(hp):
                # next head-pair's k+q projections EARLY (rope chains must
                # clear the queues well before the phase transition), with
                # whole-row ropes after each fc's last nt; V quads last
                fillers = []
                if hp < 3:
                    fillers += [load_unit(4 + hp + 1, hp + 1)]
                    for nt in range(4):
                        fillers += qkp_units(4 + hp + 1, nt, whole_rope=(nt == 3))
                    for nt in range(4):
                        fillers += qkp_units(hp + 1, nt, whole_rope=(nt == 3))
                    fillers += [vp_unit(hp + 1, (2 * i, 2 * i + 1))
                                for i in range(8)]
                return fillers

            seq = [(hp, qh) for hp in range(4) for qh in range(QH)]
            cur = make_iter(0, 0)
            cur[0](0)       # first score group right after k/q nt0 ropes
            # V(hp0) lt0-7 + remaining consts overlap with live exp now
            for lt in range(0, 8, 2):
                v_proj(lt, 0)
                v_proj(lt + 1, 0)
            const_tail()
            pending_div = None
            fillers = phase_fillers(0)
            for idx, (hp, qh) in enumerate(seq):
                if qh == 0 and hp > 0:
                    while fillers:          # flush phase leftovers
                        fillers.pop(0)()
                    fillers = phase_fillers(hp)
                nxt = make_iter(*seq[idx + 1]) if idx + 1 < len(seq) else None
                extra = []
                if hp == 0 and qh == 0:
                    # k nt2/nt3 chains must stay ~2 groups ahead of their
                    # s_groups; V(hp0) lt8-15 before this iteration's pv runs
                    extra = [qkp_unit(4, 2),
                             qkp_unit(4, 3),
                             vp_unit(0, (8, 9, 10, 11)),
                             vp_unit(0, (12, 13, 14, 15)),
                             qkp_unit(0, 1)]
                elif hp == 0 and qh == 1:
                    extra = [load_unit(5, 1), qkp_unit(0, 2)]
                elif hp == 0 and qh == 2:
                    extra = [qkp_unit(0, 3)]
                cap = 8 if hp == 3 else 4
                pending_div = iteration(cur, nxt, fillers, extra, pending_div,
                                        cap=cap)
                if hp == 3:
                    # out-projection for the finished q half-chunk
                    # (consumed next iteration, after its deferred TR)
                    qc, half = divmod(qh, 2)
                    for m in range(2):
                        fillers.extend(op_units(qc, half * 2 + m))
                cur = nxt
            if pending_div is not None:
                pending_div()
            while fillers:
                fillers.pop(0)()
    nc.compile()
    return nc


def _rope_tables_np():
    inv_freq = 1.0 / (10000.0 ** (np.arange(0, HD, 2, dtype=np.float32) / HD))
    t = np.arange(L, dtype=np.float32)
    freqs = np.outer(t, inv_freq).astype(np.float32)       # [L, 32]
    cos_h = np.cos(freqs).T                                # [32, L]
    sin_h = np.sin(freqs).T
    cosT = np.concatenate([cos_h, cos_h], 0)               # [64, L]
    sinT = np.concatenate([-sin_h, sin_h], 0)              # sign baked for rot trick
    return np.tile(cosT, (2, 1)), np.tile(sinT, (2, 1))    # [128, L] (2 heads/tile)


_NC_CACHE = {}


def kernel(x, w_qkv, b_qkv, w_out, b_out):
    import ml_dtypes
    bf16 = ml_dtypes.bfloat16
    if "nc" not in _NC_CACHE:
        _NC_CACHE["nc"] = build_nc()
    nc = _NC_CACHE["nc"]

    cosT, sinT = _rope_tables_np()
    cosT = cosT.astype(bf16)
    sinT = sinT.astype(bf16)
    in_maps = []
    for c in range(8):
        b, g = divmod(c, 2)
        s = slice(512 * g, 512 * (g + 1))
        wqk = np.concatenate([w_qkv[0:D][s], w_qkv[D:2 * D][s]], 0)  # [1024, 1024]
        in_maps.append({
            "xT": np.ascontiguousarray(x[b].T).astype(bf16),
            "wqkT": np.ascontiguousarray(
                wqk.T.reshape(8, 128, 8, 128).transpose(2, 1, 0, 3)).astype(bf16),
            "wvT": np.ascontiguousarray(w_qkv[2 * D:3 * D][s].T).astype(bf16),
            "bqk": np.ascontiguousarray(
                np.concatenate([b_qkv[0:D][s], b_qkv[D:2 * D][s]])
                .reshape(8, 128).T).astype(np.float32),
            "bv": b_qkv[2 * D:3 * D][s][None].astype(bf16),
            "woT": np.ascontiguousarray(w_out[:, s].T).astype(bf16),
            "bout": (b_out if g == 0 else np.zeros_like(b_out))[None].astype(bf16),
            "cosT": cosT,
            "sinT": sinT,
        })
    res = run_bass_kernel_spmd(nc, in_maps, list(range(8)))
    _NC_CACHE["last_results"] = res
    parts = [r["out"] for r in res.results]
    return np.stack([parts[2 * b] + parts[2 * b + 1] for b in range(4)]).astype(np.float32)


# revision 73
# speedup vs baseline: 1.0001x; 1.0001x over previous
"""Trainium2 Bass kernel for multi-head attention (B=4, L=2048, D=1024, H=16).

Sharding: 8 cores = 4 batches x 2 head-groups (8 heads each).
v2: head-pair-outer schedule with projections interleaved as fillers so the
ACT engine (exp, the ~254us floor) starts at ~10us instead of ~100us, and a
transposed PV matmul (out[qpos, feat], N=65) that halves PV's PE cost; the
softmax denominator rides along as a ones-column (per-partition scalar after
transposition), and attn output reaches the feature-major layout the output
projection needs via xbar DMA transposes on the otherwise-idle DMA engines.

All matmul operands bf16 (fp32 PSUM accumulation); output fp32.
"""
import sys

sys.path.insert(0, "/opt/trn_rl_repo")
import numpy as np
import concourse.bass as bass
import concourse.bacc as bacc
import concourse.mybir as mybir
from concourse.tile import TileContext
from concourse.bass_utils import run_bass_kernel_spmd

L = 2048          # sequence length
D = 1024          # model dim
HD = 64           # head dim
F_V = 512         # v features per core
KT = L // 128     # 16 k position tiles
QH = 8            # q half-chunks of 256
DT = mybir.dt.bfloat16
F32 = mybir.dt.float32
SCALE = HD ** -0.5
AF = mybir.ActivationFunctionType
GROUPS = [(0, 1, 2, 3), (4, 5, 6, 7), (8, 9, 10, 11), (12, 13, 14, 15)]


def build_nc():
    nc = bacc.Bacc("TRN2", target_bir_lowering=False, debug=False, num_devices=8)
    xT = nc.dram_tensor("xT", [D, L], DT, kind="ExternalInput")
    wqkT = nc.dram_tensor("wqkT", [8, 128, 8, 128], DT, kind="ExternalInput")
    wvT = nc.dram_tensor("wvT", [D, F_V], DT, kind="ExternalInput")
    bqk = nc.dram_tensor("bqk", [128, 8], F32, kind="ExternalInput")
    bv = nc.dram_tensor("bv", [1, F_V], DT, kind="ExternalInput")
    woT = nc.dram_tensor("woT", [F_V, D], DT, kind="ExternalInput")
    bout = nc.dram_tensor("bout", [1, D], DT, kind="ExternalInput")
    cosT = nc.dram_tensor("cosT", [128, L], DT, kind="ExternalInput")
    sinT = nc.dram_tensor("sinT", [128, L], DT, kind="ExternalInput")
    out = nc.dram_tensor("out", [L, D], F32, kind="ExternalOutput")

    with TileContext(nc) as tc:
        with (
            tc.tile_pool(name="const", bufs=1) as cp,
            tc.tile_pool(name="wstream", bufs=2) as wsp,
            tc.tile_pool(name="rope", bufs=2) as rp,
            tc.tile_pool(name="exps", bufs=1) as ep,
            tc.tile_pool(name="small", bufs=4) as sp,
            tc.tile_pool(name="psum", bufs=1, space="PSUM") as pp,
        ):
            dma = nc.sync
            dma2 = nc.gpsimd

            # ---- resident inputs ----
            # wqk for k of hp0 first (header need), then x chunks on both
            # queues; later fcs stream in as fillers.
            wqk_tiles = {}

            def wqk_load(fc):
                wqk_tiles[fc] = wsp.tile([128, 8, 128], DT, tag="wqk",
                                         name=f"wqk{fc}", bufs=3)
                dma.dma_start(out=wqk_tiles[fc][:], in_=wqkT[fc])

            # load order tuned for the header critical path: x half-L
            # slices split across both DMA queues (first QKPs need only the
            # first halves), rope tables tiny-first on the Pool queue, x
            # half1 and bulk weights emitted mid-header after the first
            # rope copies
            wqk_load(4)
            xT_sb = cp.tile([128, 8, L], DT)        # x.T  [d-chunk, chunk, l]
            wvT_sb = cp.tile([128, 8, F_V], DT)
            cos_sb = cp.tile([128, L], DT)
            dma2.dma_start(out=cos_sb[:], in_=cosT[:])
            sin_sb = cp.tile([128, L], DT)
            dma2.dma_start(out=sin_sb[:], in_=sinT[:])
            bqk_sb = cp.tile([128, 8], F32)
            dma2.dma_start(out=bqk_sb[:], in_=bqk[:])
            bv_sb = cp.tile([1, F_V], DT)
            dma2.dma_start(out=bv_sb[:], in_=bv[:])

            def x_load(h):
                # half0 split across both queues (it paces the header's
                # first projections); half1 all-sync so it never collides
                # with the gpsimd-side rope copies
                hs = slice(h * 1024, (h + 1) * 1024)
                for c in range(8):
                    q = dma2 if (h == 0 and c % 2) else dma
                    q.dma_start(out=xT_sb[:, c, hs],
                                in_=xT[c * 128:(c + 1) * 128, hs])

            x_load(0)
            wqk_load(0)
            bout_sb = cp.tile([1, D], DT)
            woT_sb = cp.tile([128, 4, D], DT)
            bv_bc = cp.tile([128, F_V], DT)
            bout_bc = cp.tile([128, D], DT)

            def wv_load():
                for c in range(4):
                    dma.dma_start(out=wvT_sb[:, c, :],
                                  in_=wvT[c * 128:(c + 1) * 128, :])
                for c in range(4, 8):
                    dma2.dma_start(out=wvT_sb[:, c, :],
                                   in_=wvT[c * 128:(c + 1) * 128, :])
                nc.gpsimd.partition_broadcast(bv_bc[:], bv_sb[:])

            def const_tail():
                dma.dma_start(out=bout_sb[:], in_=bout[:])
                for c in range(4):
                    dma.dma_start(out=woT_sb[:, c, :],
                                  in_=woT[c * 128:(c + 1) * 128, :])
                nc.gpsimd.partition_broadcast(bout_bc[:], bout_sb[:])

            # q (c 0-3) / k (c 4-7), feature-major; separate tiles per c so
            # late rope writes never alias earlier scores' reads
            qk_c = [cp.tile([128, L], DT, name=f"qk{c}") for c in range(8)]
            # V: [kpos, kt, hp, vA(64) 1A vB(64) 1B]; ones col = softmax denom
            V_sb = cp.tile([128, KT, 4, 130], DT)
            ones_cols = V_sb.rearrange("p t h (b c) -> p t h b c", c=65)[:, :, :, :, 64]
            nc.vector.memset(ones_cols, 1.0)
            # attn output, feature-major, per q-chunk; alive all run
            cT4 = [cp.tile([128, 4, 512], DT, name=f"cT{qc}") for qc in range(4)]

            # ---- work units ----
            def qk_proj(fc, nt, ptag="pp"):
                if fc in wqk_tiles:
                    wqk_t = wqk_tiles[fc]
                else:
                    wqk_load(fc)
                    wqk_t = wqk_tiles[fc]
                qps = pp.tile([128, 512], F32, tag=ptag, name=f"qps{fc}_{nt}",
                              bufs=2 if ptag == "pp" else 1)
                for kc in range(8):
                    nc.tensor.matmul(qps[:], lhsT=wqk_t[:, kc, :],
                                     rhs=xT_sb[:, kc, nt * 512:(nt + 1) * 512],
                                     start=(kc == 0), stop=(kc == 7))
                nc.vector.tensor_scalar_add(qk_c[fc][:, nt * 512:(nt + 1) * 512],
                                            qps[:], bqk_sb[:, fc:fc + 1])

            def rope(c, nt=None):
                # copies split across the two DMA queues; arithmetic on DVE
                # (Pool's tensor ops are 3x slower, no 2x bf16 mode).
                # nt=None ropes the whole row (filler path, 4x fewer DMAs)
                if nt is None:
                    s = slice(0, L)
                    w, rtag, ttag = L, "rotW", "rtmpW"
                else:
                    s = slice(nt * 512, (nt + 1) * 512)
                    w, rtag, ttag = 512, "rot", "rtmp"
                rot = rp.tile([128, w], DT, tag=rtag, name=f"rot{c}_{nt}")
                for h2 in range(2):
                    p = 64 * h2
                    (dma, dma2)[h2].dma_start(out=rot[p:p + 32, :],
                                              in_=qk_c[c][p + 32:p + 64, s])
                    (dma2, dma)[h2].dma_start(out=rot[p + 32:p + 64, :],
                                              in_=qk_c[c][p:p + 32, s])
                tmp = rp.tile([128, w], DT, tag=ttag, name=f"rtmp{c}_{nt}")
                nc.vector.tensor_mul(tmp[:], qk_c[c][:, s], cos_sb[:, s])
                nc.vector.tensor_mul(rot[:], rot[:], sin_sb[:, s])
                nc.vector.tensor_add(qk_c[c][:, s], tmp[:], rot[:])

            def v_proj(lt, hp):
                vps = pp.tile([128, 128], F32, tag="pp", bufs=2,
                              name=f"vps{lt}_{hp}")
                for kc in range(8):
                    nc.tensor.matmul(vps[:],
                                     lhsT=xT_sb[:, kc, lt * 128:(lt + 1) * 128],
                                     rhs=wvT_sb[:, kc, hp * 128:(hp + 1) * 128],
                                     start=(kc == 0), stop=(kc == 7))
                for h2 in range(2):
                    nc.vector.tensor_add(
                        V_sb[:, lt, hp, h2 * 65:h2 * 65 + 64],
                        vps[:, h2 * 64:(h2 + 1) * 64],
                        bv_bc[:, hp * 128 + h2 * 64:hp * 128 + (h2 + 1) * 64])

            def out_proj_half(qc, mq, dt_, osb):
                # one d-half per filler slot (0.85us fits the inter-group
                # PE budget); merged [128, 1024] store after the second half
                ops = pp.tile([128, 512], F32, tag="pp", bufs=2,
                              name=f"ops{qc}_{dt_}_{mq}")
                for cc in range(4):
                    nc.tensor.matmul(ops[:],
                                     lhsT=cT4[qc][:, cc, mq * 128:(mq + 1) * 128],
                                     rhs=woT_sb[:, cc, dt_ * 512:(dt_ + 1) * 512],
                                     start=(cc == 0), stop=(cc == 3))
                nc.vector.tensor_add(osb[:, dt_ * 512:(dt_ + 1) * 512], ops[:],
                                     bout_bc[:, dt_ * 512:(dt_ + 1) * 512])
                if dt_ == 1:
                    dma.dma_start(
                        out=out[qc * 512 + mq * 128: qc * 512 + (mq + 1) * 128, :],
                        in_=osb[:])

            # ---- attention iteration state (hp, qh): 256 q positions,
            # 2 heads; software-pipelined: S(i+1, g0) is emitted before
            # iteration i's pv runs so ACT never waits out the PV block
            def make_iter(hp, qh):
                qs = slice(qh * 256, (qh + 1) * 256)
                expA = ep.tile([128, KT, 256], DT, tag="expA", bufs=2,
                               name=f"expA{hp}_{qh}")
                expB = ep.tile([128, KT, 256], DT, tag="expB", bufs=2,
                               name=f"expB{hp}_{qh}")
                pv = pp.tile([128, 2, 130], F32, tag="pv", bufs=2,
                             name=f"pv{hp}_{qh}")

                def s_group(gi):
                    kts = GROUPS[gi]
                    spsA = pp.tile([128, 1024], F32, tag="sA", name=f"sA{hp}_{qh}_{gi}")
                    spsB = pp.tile([128, 1024], F32, tag="sB", name=f"sB{hp}_{qh}_{gi}")
                    for j, kt in enumerate(kts):
                        for p, sps in ((0, spsA), (64, spsB)):
                            nc.tensor.matmul(
                                sps[:, j * 256:(j + 1) * 256],
                                lhsT=qk_c[4 + hp][p:p + 64, kt * 128:(kt + 1) * 128],
                                rhs=qk_c[hp][p:p + 64, qs],
                                start=True, stop=True)
                    n = len(kts)
                    k0 = kts[0]
                    nc.scalar.activation(
                        expA[:, k0:k0 + n, :].rearrange("p a b -> p (a b)"),
                        spsA[:, 0:n * 256], AF.Exp, scale=SCALE)
                    nc.scalar.activation(
                        expB[:, k0:k0 + n, :].rearrange("p a b -> p (a b)"),
                        spsB[:, 0:n * 256], AF.Exp, scale=SCALE)

                def pv_run(m, h2):
                    # one full contiguous accumulation run: interleaving
                    # different accumulation groups within one PSUM bank
                    # corrupts results (other-bank matmuls between runs are
                    # fine)
                    expS = (expA, expB)[h2]
                    for kt in range(KT):
                        nc.tensor.matmul(
                            pv[:, m, h2 * 65:(h2 + 1) * 65],
                            lhsT=expS[:, kt, m * 128:(m + 1) * 128],
                            rhs=V_sb[:, kt, hp, h2 * 65:(h2 + 1) * 65],
                            start=(kt == 0), stop=(kt == KT - 1))

                def div_m(m):
                    # divide by denominator (ones-column), cast bf16, transpose
                    rr = sp.tile([128, 2], F32, tag="rr", bufs=4,
                                 name=f"rr{hp}_{qh}_{m}")
                    nc.vector.reciprocal(
                        rr[:], pv[:, m].rearrange("p (h c) -> p h c", c=65)[:, :, 64])
                    qc, half = divmod(qh, 2)
                    aq = sp.tile([128, 128], DT, tag="aq", bufs=4,
                                 name=f"aq{hp}_{qh}_{m}")
                    for h2 in range(2):
                        nc.vector.tensor_scalar_mul(
                            aq[:, h2 * 64:(h2 + 1) * 64],
                            pv[:, m, h2 * 65:h2 * 65 + 64],
                            rr[:, h2:h2 + 1])
                    off = half * 256 + m * 128
                    dma.dma_start_transpose(out=cT4[qc][:, hp, off:off + 128],
                                            in_=aq[:])

                def div_tr():
                    div_m(0)
                    div_m(1)
                return s_group, pv_run, div_tr, div_m

            def iteration(cur, nxt, fillers, extra=(), pending_div=None, cap=2):
                # cur = (s_group, pv_run, div_tr); cur's s_group(0) was
                # already emitted by the previous iteration
                s_group, pv_run = cur[0], cur[1]
                if pending_div is not None:
                    # previous iteration's divide+transpose, deferred so the
                    # DVE reciprocal never head-of-line-blocks behind a
                    # still-accumulating pv bank, and the SP-queue transpose
                    # never waits on a fresh ts_mul
                    pending_div()
                extra = list(extra)
                budget = [cap]

                def fill(n=1):
                    # extras are pinned to this iteration; rationed fillers
                    # spread across the phase so late iterations keep PE
                    # cover during exp waits
                    for _ in range(n):
                        if extra:
                            extra.pop(0)()
                        elif fillers and budget[0] > 0:
                            budget[0] -= 1
                            fillers.pop(0)()

                fill()
                s_group(1)
                fill()
                s_group(2)
                fill()
                s_group(3)
                fill()
                if nxt is not None:
                    nxt[0](0)      # pipelined S(i+1, g0)
                pv_run(0, 0)
                pv_run(0, 1)
                fill()
                if nxt is None:
                    # drain fast: m0's divide/transpose overlaps m1's pv
                    cur[3](0)
                pv_run(1, 0)
                pv_run(1, 1)
                fill()
                if nxt is None:
                    cur[3](1)
                    return None
                return cur[2]

            # ---- schedule ----
            # header: k(hp0) nt0-2, q(hp0) nt0 + ropes
            # ((4,3) rides as an it0 extra so S(g0) isn't pushed back)
            qk_proj(4, 0, "sA")
            rope(4, 0)
            qk_proj(0, 0, "sB")
            rope(0, 0)
            qk_proj(4, 1, "pp")
            rope(4, 1)
            wv_load()
            x_load(1)

            def qkp_unit(fc, nt, do_rope=True, whole_rope=False):
                def emit():
                    qk_proj(fc, nt)
                    if whole_rope:
                        rope(fc)        # whole-L rope after the last nt
                    elif do_rope:
                        rope(fc, nt)
                return emit

            def qkp_units(fc, nt, whole_rope=False):
                # two 0.85us halves of the 8-matmul contraction; the psum
                # accumulation group safely spans other-bank matmuls
                state = {}

                def emit0():
                    if fc not in wqk_tiles:
                        wqk_load(fc)
                    state["qps"] = pp.tile([128, 512], F32, tag="pp", bufs=2,
                                           name=f"qps{fc}_{nt}")
                    for kc in range(4):
                        nc.tensor.matmul(state["qps"][:],
                                         lhsT=wqk_tiles[fc][:, kc, :],
                                         rhs=xT_sb[:, kc, nt * 512:(nt + 1) * 512],
                                         start=(kc == 0), stop=False)

                def emit1():
                    for kc in range(4, 8):
                        nc.tensor.matmul(state["qps"][:],
                                         lhsT=wqk_tiles[fc][:, kc, :],
                                         rhs=xT_sb[:, kc, nt * 512:(nt + 1) * 512],
                                         start=False, stop=(kc == 7))
                    nc.vector.tensor_scalar_add(
                        qk_c[fc][:, nt * 512:(nt + 1) * 512],
                        state["qps"][:], bqk_sb[:, fc:fc + 1])
                    if whole_rope:
                        rope(fc)
                return [emit0, emit1]

            def load_unit(*fcs):
                def emit():
                    for fc in fcs:
                        if fc not in wqk_tiles:
                            wqk_load(fc)
                return emit

            def vp_unit(hp, lts):
                def emit():
                    for lt in lts:
                        v_proj(lt, hp)
                return emit

            def op_units(qc, mq):
                state = {}

                def emit0():
                    state["osb"] = sp.tile([128, D], F32, tag="osb", bufs=2,
                                           name=f"osb{qc}_{mq}")
                    out_proj_half(qc, mq, 0, state["osb"])

                def emit1():
                    out_proj_half(qc, mq, 1, state["osb"])
                return [emit0, emit1]

            def phase_fillers(hp):
                # next head-pair's k+q projections EARLY (rope chains must
                # clear the queues well before the phase transition), with
                # whole-row ropes after each fc's last nt; V quads last
                fillers = []
                if hp < 3:
                    fillers += [load_unit(4 + hp + 1, hp + 1)]
                    for nt in range(4):
                        fillers += qkp_units(4 + hp + 1, nt, whole_rope=(nt == 3))
                    for nt in range(4):
                        fillers += qkp_units(hp + 1, nt, whole_rope=(nt == 3))
                    fillers += [vp_unit(hp + 1, (2 * i, 2 * i + 1))
                                for i in range(8)]
                return fillers

            seq = [(hp, qh) for hp in range(4) for qh in range(QH)]
            cur = make_iter(0, 0)
            cur[0](0)       # first score group right after k/q nt0 ropes
            # V(hp0) lt0-7 + remaining consts overlap with live exp now
            for lt in range(0, 8, 2):
                v_proj(lt, 0)
                v_proj(lt + 1, 0)
            const_tail()
            pending_div = None
            fillers = phase_fillers(0)
            for idx, (hp, qh) in enumerate(seq):
                if qh == 0 and hp > 0:
                    while fillers:          # flush phase leftovers
                        fillers.pop(0)()
                    fillers = phase_fillers(hp)
                nxt = make_iter(*seq[idx + 1]) if idx + 1 < len(seq) else None
                extra = []
                if hp == 0 and qh == 0:
                    # k nt2/nt3 chains must stay ~2 groups ahead of their
                    # s_groups; V(hp0) lt8-15 before this iteration's pv runs
                    extra = [qkp_unit(4, 2),
                             qkp_unit(4, 3),
                             vp_unit(0, (8, 9, 10, 11)),
                             vp_unit(0, (12, 13, 14, 15)),
                             qkp_unit(0, 1)]
                elif hp == 0 and qh == 1:
                    extra = [load_unit(5, 1), qkp_unit(0, 2)]
                elif hp == 0 and qh == 2:
                    extra = [qkp_unit(0, 3)]
                cap = 8 if hp == 3 else 4
                pending_div = iteration(cur, nxt, fillers, extra, pending_div,
                                        cap=cap)
                if hp == 3:
                    # out-projection for the finished q half-chunk
                    # (consumed next iteration, after its deferred TR)
                    qc, half = divmod(qh, 2)
                    for m in range(2):
                        fillers.extend(op_units(qc, half * 2 + m))
                cur = nxt
            if pending_div is not None:
                pending_div()
            while fillers:
                fillers.pop(0)()
    nc.compile()
    return nc


def _rope_tables_np():
    inv_freq = 1.0 / (10000.0 ** (np.arange(0, HD, 2, dtype=np.float32) / HD))
    t = np.arange(L, dtype=np.float32)
    freqs = np.outer(t, inv_freq).astype(np.float32)       # [L, 32]
    cos_h = np.cos(freqs).T                                # [32, L]
    sin_h = np.sin(freqs).T
    cosT = np.concatenate([cos_h, cos_h], 0)               # [64, L]
    sinT = np.concatenate([-sin_h, sin_h], 0)              # sign baked for rot trick
    return np.tile(cosT, (2, 1)), np.tile(sinT, (2, 1))    # [128, L] (2 heads/tile)


_NC_CACHE = {}


def kernel(x, w_qkv, b_qkv, w_out, b_out):
    import ml_dtypes
    bf16 = ml_dtypes.bfloat16
    if "nc" not in _NC_CACHE:
        _NC_CACHE["nc"] = build_nc()
    nc = _NC_CACHE["nc"]

    cosT, sinT = _rope_tables_np()
    cosT = cosT.astype(bf16)
    sinT = sinT.astype(bf16)
    in_maps = []
    for c in range(8):
        b, g = divmod(c, 2)
        s = slice(512 * g, 512 * (g + 1))
        wqk = np.concatenate([w_qkv[0:D][s], w_qkv[D:2 * D][s]], 0)  # [1024, 1024]
        in_maps.append({
            "xT": np.ascontiguousarray(x[b].T).astype(bf16),
            "wqkT": np.ascontiguousarray(
                wqk.T.reshape(8, 128, 8, 128).transpose(2, 1, 0, 3)).astype(bf16),
            "wvT": np.ascontiguousarray(w_qkv[2 * D:3 * D][s].T).astype(bf16),
            "bqk": np.ascontiguousarray(
                np.concatenate([b_qkv[0:D][s], b_qkv[D:2 * D][s]])
                .reshape(8, 128).T).astype(np.float32),
            "bv": b_qkv[2 * D:3 * D][s][None].astype(bf16),
            "woT": np.ascontiguousarray(w_out[:, s].T).astype(bf16),
            "bout": (b_out if g == 0 else np.zeros_like(b_out))[None].astype(bf16),
            "cosT": cosT,
            "sinT": sinT,
        })
    res = run_bass_kernel_spmd(nc, in_maps, list(range(8)))
    _NC_CACHE["last_results"] = res
    parts = [r["out"] for r in res.results]
    return np.stack([parts[2 * b] + parts[2 * b + 1] for b in range(4)]).astype(np.float32)


# revision 74
# speedup vs baseline: 1.0102x; 1.0102x over previous
"""Trainium2 Bass kernel for multi-head attention (B=4, L=2048, D=1024, H=16).

Sharding: 8 cores = 4 batches x 2 head-groups (8 heads each).
v2: head-pair-outer schedule with projections interleaved as fillers so the
ACT engine (exp, the ~254us floor) starts at ~10us instead of ~100us, and a
transposed PV matmul (out[qpos, feat], N=65) that halves PV's PE cost; the
softmax denominator rides along as a ones-column (per-partition scalar after
transposition), and attn output reaches the feature-major layout the output
projection needs via xbar DMA transposes on the otherwise-idle DMA engines.

All matmul operands bf16 (fp32 PSUM accumulation); output fp32.
"""
import sys

sys.path.insert(0, "/opt/trn_rl_repo")
import numpy as np
import concourse.bass as bass
import concourse.bacc as bacc
import concourse.mybir as mybir
from concourse.tile import TileContext
from concourse.bass_utils import run_bass_kernel_spmd

L = 2048          # sequence length
D = 1024          # model dim
HD = 64           # head dim
F_V = 512         # v features per core
KT = L // 128     # 16 k position tiles
QH = 8            # q half-chunks of 256
DT = mybir.dt.bfloat16
F32 = mybir.dt.float32
SCALE = HD ** -0.5
AF = mybir.ActivationFunctionType
GROUPS = [(0, 1, 2, 3), (4, 5, 6, 7), (8, 9, 10, 11), (12, 13, 14, 15)]


def build_nc():
    nc = bacc.Bacc("TRN2", target_bir_lowering=False, debug=False, num_devices=8)
    xT = nc.dram_tensor("xT", [D, L], DT, kind="ExternalInput")
    wqkT = nc.dram_tensor("wqkT", [8, 128, 8, 128], DT, kind="ExternalInput")
    wvT = nc.dram_tensor("wvT", [D, F_V], DT, kind="ExternalInput")
    bqk = nc.dram_tensor("bqk", [128, 8], F32, kind="ExternalInput")
    bv = nc.dram_tensor("bv", [1, F_V], DT, kind="ExternalInput")
    woT = nc.dram_tensor("woT", [F_V, D], DT, kind="ExternalInput")
    bout = nc.dram_tensor("bout", [1, D], DT, kind="ExternalInput")
    cosT = nc.dram_tensor("cosT", [128, L], DT, kind="ExternalInput")
    sinT = nc.dram_tensor("sinT", [128, L], DT, kind="ExternalInput")
    out = nc.dram_tensor("out", [L, D], F32, kind="ExternalOutput")

    with TileContext(nc) as tc:
        with (
            tc.tile_pool(name="const", bufs=1) as cp,
            tc.tile_pool(name="wstream", bufs=2) as wsp,
            tc.tile_pool(name="rope", bufs=2) as rp,
            tc.tile_pool(name="exps", bufs=1) as ep,
            tc.tile_pool(name="small", bufs=4) as sp,
            tc.tile_pool(name="psum", bufs=1, space="PSUM") as pp,
        ):
            dma = nc.sync
            dma2 = nc.gpsimd

            # ---- resident inputs ----
            # wqk for k of hp0 first (header need), then x chunks on both
            # queues; later fcs stream in as fillers.
            wqk_tiles = {}

            def wqk_load(fc):
                wqk_tiles[fc] = wsp.tile([128, 8, 128], DT, tag="wqk",
                                         name=f"wqk{fc}", bufs=3)
                dma.dma_start(out=wqk_tiles[fc][:], in_=wqkT[fc])

            # load order tuned for the header critical path: x half-L
            # slices split across both DMA queues (first QKPs need only the
            # first halves), rope tables tiny-first on the Pool queue, x
            # half1 and bulk weights emitted mid-header after the first
            # rope copies
            wqk_load(4)
            xT_sb = cp.tile([128, 8, L], DT)        # x.T  [d-chunk, chunk, l]
            wvT_sb = cp.tile([128, 8, F_V], DT)
            cos_sb = cp.tile([128, L], DT)
            dma2.dma_start(out=cos_sb[:], in_=cosT[:])
            sin_sb = cp.tile([128, L], DT)
            dma2.dma_start(out=sin_sb[:], in_=sinT[:])
            bqk_sb = cp.tile([128, 8], F32)
            dma2.dma_start(out=bqk_sb[:], in_=bqk[:])
            bv_sb = cp.tile([1, F_V], DT)
            dma2.dma_start(out=bv_sb[:], in_=bv[:])

            def x_load(h):
                # all on the sync queue; half1 is emitted mid-header so the
                # first rope copies jump ahead of it in the queue
                hs = slice(h * 1024, (h + 1) * 1024)
                for c in range(8):
                    dma.dma_start(out=xT_sb[:, c, hs],
                                  in_=xT[c * 128:(c + 1) * 128, hs])

            x_load(0)
            wqk_load(0)
            bout_sb = cp.tile([1, D], DT)
            woT_sb = cp.tile([128, 4, D], DT)
            bv_bc = cp.tile([128, F_V], DT)
            bout_bc = cp.tile([128, D], DT)

            def wv_load():
                for c in range(4):
                    dma.dma_start(out=wvT_sb[:, c, :],
                                  in_=wvT[c * 128:(c + 1) * 128, :])
                for c in range(4, 8):
                    dma2.dma_start(out=wvT_sb[:, c, :],
                                   in_=wvT[c * 128:(c + 1) * 128, :])
                nc.gpsimd.partition_broadcast(bv_bc[:], bv_sb[:])

            def const_tail():
                dma.dma_start(out=bout_sb[:], in_=bout[:])
                for c in range(4):
                    dma.dma_start(out=woT_sb[:, c, :],
                                  in_=woT[c * 128:(c + 1) * 128, :])
                nc.gpsimd.partition_broadcast(bout_bc[:], bout_sb[:])

            # q (c 0-3) / k (c 4-7), feature-major; separate tiles per c so
            # late rope writes never alias earlier scores' reads
            qk_c = [cp.tile([128, L], DT, name=f"qk{c}") for c in range(8)]
            # V: [kpos, kt, hp, vA(64) 1A vB(64) 1B]; ones col = softmax denom
            V_sb = cp.tile([128, KT, 4, 130], DT)
            ones_cols = V_sb.rearrange("p t h (b c) -> p t h b c", c=65)[:, :, :, :, 64]
            nc.vector.memset(ones_cols, 1.0)
            # attn output, feature-major, per q-chunk; alive all run
            cT4 = [cp.tile([128, 4, 512], DT, name=f"cT{qc}") for qc in range(4)]

            # ---- work units ----
            def qk_proj(fc, nt, ptag="pp"):
                if fc in wqk_tiles:
                    wqk_t = wqk_tiles[fc]
                else:
                    wqk_load(fc)
                    wqk_t = wqk_tiles[fc]
                qps = pp.tile([128, 512], F32, tag=ptag, name=f"qps{fc}_{nt}",
                              bufs=2 if ptag == "pp" else 1)
                for kc in range(8):
                    nc.tensor.matmul(qps[:], lhsT=wqk_t[:, kc, :],
                                     rhs=xT_sb[:, kc, nt * 512:(nt + 1) * 512],
                                     start=(kc == 0), stop=(kc == 7))
                nc.vector.tensor_scalar_add(qk_c[fc][:, nt * 512:(nt + 1) * 512],
                                            qps[:], bqk_sb[:, fc:fc + 1])

            def rope(c, nt=None):
                # copies split across the two DMA queues; arithmetic on DVE
                # (Pool's tensor ops are 3x slower, no 2x bf16 mode).
                # nt=None ropes the whole row (filler path, 4x fewer DMAs)
                if nt is None:
                    s = slice(0, L)
                    w, rtag, ttag = L, "rotW", "rtmpW"
                else:
                    s = slice(nt * 512, (nt + 1) * 512)
                    w, rtag, ttag = 512, "rot", "rtmp"
                rot = rp.tile([128, w], DT, tag=rtag, name=f"rot{c}_{nt}")
                for h2 in range(2):
                    p = 64 * h2
                    (dma, dma2)[h2].dma_start(out=rot[p:p + 32, :],
                                              in_=qk_c[c][p + 32:p + 64, s])
                    (dma2, dma)[h2].dma_start(out=rot[p + 32:p + 64, :],
                                              in_=qk_c[c][p:p + 32, s])
                tmp = rp.tile([128, w], DT, tag=ttag, name=f"rtmp{c}_{nt}")
                nc.vector.tensor_mul(tmp[:], qk_c[c][:, s], cos_sb[:, s])
                nc.vector.tensor_mul(rot[:], rot[:], sin_sb[:, s])
                nc.vector.tensor_add(qk_c[c][:, s], tmp[:], rot[:])

            def v_proj(lt, hp):
                vps = pp.tile([128, 128], F32, tag="pp", bufs=2,
                              name=f"vps{lt}_{hp}")
                for kc in range(8):
                    nc.tensor.matmul(vps[:],
                                     lhsT=xT_sb[:, kc, lt * 128:(lt + 1) * 128],
                                     rhs=wvT_sb[:, kc, hp * 128:(hp + 1) * 128],
                                     start=(kc == 0), stop=(kc == 7))
                for h2 in range(2):
                    nc.vector.tensor_add(
                        V_sb[:, lt, hp, h2 * 65:h2 * 65 + 64],
                        vps[:, h2 * 64:(h2 + 1) * 64],
                        bv_bc[:, hp * 128 + h2 * 64:hp * 128 + (h2 + 1) * 64])

            def out_proj_half(qc, mq, dt_, osb):
                # one d-half per filler slot (0.85us fits the inter-group
                # PE budget); merged [128, 1024] store after the second half
                ops = pp.tile([128, 512], F32, tag="pp", bufs=2,
                              name=f"ops{qc}_{dt_}_{mq}")
                for cc in range(4):
                    nc.tensor.matmul(ops[:],
                                     lhsT=cT4[qc][:, cc, mq * 128:(mq + 1) * 128],
                                     rhs=woT_sb[:, cc, dt_ * 512:(dt_ + 1) * 512],
                                     start=(cc == 0), stop=(cc == 3))
                nc.vector.tensor_add(osb[:, dt_ * 512:(dt_ + 1) * 512], ops[:],
                                     bout_bc[:, dt_ * 512:(dt_ + 1) * 512])
                if dt_ == 1:
                    dma.dma_start(
                        out=out[qc * 512 + mq * 128: qc * 512 + (mq + 1) * 128, :],
                        in_=osb[:])

            # ---- attention iteration state (hp, qh): 256 q positions,
            # 2 heads; software-pipelined: S(i+1, g0) is emitted before
            # iteration i's pv runs so ACT never waits out the PV block
            def make_iter(hp, qh):
                qs = slice(qh * 256, (qh + 1) * 256)
                expA = ep.tile([128, KT, 256], DT, tag="expA", bufs=2,
                               name=f"expA{hp}_{qh}")
                expB = ep.tile([128, KT, 256], DT, tag="expB", bufs=2,
                               name=f"expB{hp}_{qh}")
                pv = pp.tile([128, 2, 130], F32, tag="pv", bufs=2,
                             name=f"pv{hp}_{qh}")

                def s_group(gi):
                    kts = GROUPS[gi]
                    spsA = pp.tile([128, 1024], F32, tag="sA", name=f"sA{hp}_{qh}_{gi}")
                    spsB = pp.tile([128, 1024], F32, tag="sB", name=f"sB{hp}_{qh}_{gi}")
                    for j, kt in enumerate(kts):
                        for p, sps in ((0, spsA), (64, spsB)):
                            nc.tensor.matmul(
                                sps[:, j * 256:(j + 1) * 256],
                                lhsT=qk_c[4 + hp][p:p + 64, kt * 128:(kt + 1) * 128],
                                rhs=qk_c[hp][p:p + 64, qs],
                                start=True, stop=True)
                    n = len(kts)
                    k0 = kts[0]
                    nc.scalar.activation(
                        expA[:, k0:k0 + n, :].rearrange("p a b -> p (a b)"),
                        spsA[:, 0:n * 256], AF.Exp, scale=SCALE)
                    nc.scalar.activation(
                        expB[:, k0:k0 + n, :].rearrange("p a b -> p (a b)"),
                        spsB[:, 0:n * 256], AF.Exp, scale=SCALE)

                def pv_run(m, h2):
                    # one full contiguous accumulation run: interleaving
                    # different accumulation groups within one PSUM bank
                    # corrupts results (other-bank matmuls between runs are
                    # fine)
                    expS = (expA, expB)[h2]
                    for kt in range(KT):
                        nc.tensor.matmul(
                            pv[:, m, h2 * 65:(h2 + 1) * 65],
                            lhsT=expS[:, kt, m * 128:(m + 1) * 128],
                            rhs=V_sb[:, kt, hp, h2 * 65:(h2 + 1) * 65],
                            start=(kt == 0), stop=(kt == KT - 1))

                def div_m(m):
                    # divide by denominator (ones-column), cast bf16, transpose
                    rr = sp.tile([128, 2], F32, tag="rr", bufs=4,
                                 name=f"rr{hp}_{qh}_{m}")
                    nc.vector.reciprocal(
                        rr[:], pv[:, m].rearrange("p (h c) -> p h c", c=65)[:, :, 64])
                    qc, half = divmod(qh, 2)
                    aq = sp.tile([128, 128], DT, tag="aq", bufs=4,
                                 name=f"aq{hp}_{qh}_{m}")
                    for h2 in range(2):
                        nc.vector.tensor_scalar_mul(
                            aq[:, h2 * 64:(h2 + 1) * 64],
                            pv[:, m, h2 * 65:h2 * 65 + 64],
                            rr[:, h2:h2 + 1])
                    off = half * 256 + m * 128
                    dma.dma_start_transpose(out=cT4[qc][:, hp, off:off + 128],
                                            in_=aq[:])

                def div_tr():
                    div_m(0)
                    div_m(1)
                return s_group, pv_run, div_tr, div_m

            def iteration(cur, nxt, fillers, extra=(), pending_div=None, cap=2):
                # cur = (s_group, pv_run, div_tr); cur's s_group(0) was
                # already emitted by the previous iteration
                s_group, pv_run = cur[0], cur[1]
                if pending_div is not None:
                    # previous iteration's divide+transpose, deferred so the
                    # DVE reciprocal never head-of-line-blocks behind a
                    # still-accumulating pv bank, and the SP-queue transpose
                    # never waits on a fresh ts_mul
                    pending_div()
                extra = list(extra)
                budget = [cap]

                def fill(n=1):
                    # extras are pinned to this iteration; rationed fillers
                    # spread across the phase so late iterations keep PE
                    # cover during exp waits
                    for _ in range(n):
                        if extra:
                            extra.pop(0)()
                        elif fillers and budget[0] > 0:
                            budget[0] -= 1
                            fillers.pop(0)()

                fill()
                s_group(1)
                fill()
                s_group(2)
                fill()
                s_group(3)
                fill()
                if nxt is not None:
                    nxt[0](0)      # pipelined S(i+1, g0)
                pv_run(0, 0)
                pv_run(0, 1)
                fill()
                if nxt is None:
                    # drain fast: m0's divide/transpose overlaps m1's pv
                    cur[3](0)
                pv_run(1, 0)
                pv_run(1, 1)
                fill()
                if nxt is None:
                    cur[3](1)
                    return None
                return cur[2]

            # ---- schedule ----
            # header: k(hp0) nt0-2, q(hp0) nt0 + ropes
            # ((4,3) rides as an it0 extra so S(g0) isn't pushed back)
            qk_proj(4, 0, "sA")
            rope(4, 0)
            qk_proj(0, 0, "sB")
            rope(0, 0)
            qk_proj(4, 1, "pp")
            rope(4, 1)
            wv_load()
            x_load(1)

            def qkp_unit(fc, nt, do_rope=True, whole_rope=False):
                def emit():
                    qk_proj(fc, nt)
                    if whole_rope:
                        rope(fc)        # whole-L rope after the last nt
                    elif do_rope:
                        rope(fc, nt)
                return emit

            def qkp_units(fc, nt, whole_rope=False):
                # two 0.85us halves of the 8-matmul contraction; the psum
                # accumulation group safely spans other-bank matmuls
                state = {}

                def emit0():
                    if fc not in wqk_tiles:
                        wqk_load(fc)
                    state["qps"] = pp.tile([128, 512], F32, tag="pp", bufs=2,
                                           name=f"qps{fc}_{nt}")
                    for kc in range(4):
                        nc.tensor.matmul(state["qps"][:],
                                         lhsT=wqk_tiles[fc][:, kc, :],
                                         rhs=xT_sb[:, kc, nt * 512:(nt + 1) * 512],
                                         start=(kc == 0), stop=False)

                def emit1():
                    for kc in range(4, 8):
                        nc.tensor.matmul(state["qps"][:],
                                         lhsT=wqk_tiles[fc][:, kc, :],
                                         rhs=xT_sb[:, kc, nt * 512:(nt + 1) * 512],
                                         start=False, stop=(kc == 7))
                    nc.vector.tensor_scalar_add(
                        qk_c[fc][:, nt * 512:(nt + 1) * 512],
                        state["qps"][:], bqk_sb[:, fc:fc + 1])
                    if whole_rope:
                        rope(fc)
                return [emit0, emit1]

            def load_unit(*fcs):
                def emit():
                    for fc in fcs:
                        if fc not in wqk_tiles:
                            wqk_load(fc)
                return emit

            def vp_unit(hp, lts):
                def emit():
                    for lt in lts:
                        v_proj(lt, hp)
                return emit

            def op_units(qc, mq):
                state = {}

                def emit0():
                    state["osb"] = sp.tile([128, D], F32, tag="osb", bufs=2,
                                           name=f"osb{qc}_{mq}")
                    out_proj_half(qc, mq, 0, state["osb"])

                def emit1():
                    out_proj_half(qc, mq, 1, state["osb"])
                return [emit0, emit1]

            def phase_fillers(hp):
                # next head-pair's k+q projections EARLY (rope chains must
                # clear the queues well before the phase transition), with
                # whole-row ropes after each fc's last nt; V quads last
                fillers = []
                if hp < 3:
                    fillers += [load_unit(4 + hp + 1, hp + 1)]
                    for nt in range(4):
                        fillers += qkp_units(4 + hp + 1, nt, whole_rope=(nt == 3))
                    for nt in range(4):
                        fillers += qkp_units(hp + 1, nt, whole_rope=(nt == 3))
                    fillers += [vp_unit(hp + 1, (2 * i, 2 * i + 1))
                                for i in range(8)]
                return fillers

            seq = [(hp, qh) for hp in range(4) for qh in range(QH)]
            cur = make_iter(0, 0)
            cur[0](0)       # first score group right after k/q nt0 ropes
            # V(hp0) lt0-7 + remaining consts overlap with live exp now
            for lt in range(0, 8, 2):
                v_proj(lt, 0)
                v_proj(lt + 1, 0)
            const_tail()
            pending_div = None
            fillers = phase_fillers(0)
            for idx, (hp, qh) in enumerate(seq):
                if qh == 0 and hp > 0:
                    while fillers:          # flush phase leftovers
                        fillers.pop(0)()
                    fillers = phase_fillers(hp)
                nxt = make_iter(*seq[idx + 1]) if idx + 1 < len(seq) else None
                extra = []
                if hp == 0 and qh == 0:
                    # k nt2/nt3 chains must stay ~2 groups ahead of their
                    # s_groups; V(hp0) lt8-15 before this iteration's pv runs
                    extra = [qkp_unit(4, 2),
                             qkp_unit(4, 3),
                             vp_unit(0, (8, 9, 10, 11)),
                             vp_unit(0, (12, 13, 14, 15)),
                             qkp_unit(0, 1)]
                elif hp == 0 and qh == 1:
                    extra = [load_unit(5, 1), qkp_unit(0, 2)]
                elif hp == 0 and qh == 2:
                    extra = [qkp_unit(0, 3)]
                cap = 8 if hp == 3 else 4
                pending_div = iteration(cur, nxt, fillers, extra, pending_div,
                                        cap=cap)
                if hp == 3:
                    # out-projection for the finished q half-chunk
                    # (consumed next iteration, after its deferred TR)
                    qc, half = divmod(qh, 2)
                    for m in range(2):
                        fillers.extend(op_units(qc, half * 2 + m))
                cur = nxt
            if pending_div is not None:
                pending_div()
            while fillers:
                fillers.pop(0)()
    nc.compile()
    return nc


def _rope_tables_np():
    inv_freq = 1.0 / (10000.0 ** (np.arange(0, HD, 2, dtype=np.float32) / HD))
    t = np.arange(L, dtype=np.float32)
    freqs = np.outer(t, inv_freq).astype(np.float32)       # [L, 32]
    cos_h = np.cos(freqs).T                                # [32, L]
    sin_h = np.sin(freqs).T
    cosT = np.concatenate([cos_h, cos_h], 0)               # [64, L]
    sinT = np.concatenate([-sin_h, sin_h], 0)              # sign baked for rot trick
    return np.tile(cosT, (2, 1)), np.tile(sinT, (2, 1))    # [128, L] (2 heads/tile)


_NC_CACHE = {}


def kernel(x, w_qkv, b_qkv, w_out, b_out):
    import ml_dtypes
    bf16 = ml_dtypes.bfloat16
    if "nc" not in _NC_CACHE:
        _NC_CACHE["nc"] = build_nc()
    nc = _NC_CACHE["nc"]

    cosT, sinT = _rope_tables_np()
    cosT = cosT.astype(bf16)
    sinT = sinT.astype(bf16)
    in_maps = []
    for c in range(8):
        b, g = divmod(c, 2)
        s = slice(512 * g, 512 * (g + 1))
        wqk = np.concatenate([w_qkv[0:D][s], w_qkv[D:2 * D][s]], 0)  # [1024, 1024]
        in_maps.append({
            "xT": np.ascontiguousarray(x[b].T).astype(bf16),
            "wqkT": np.ascontiguousarray(
                wqk.T.reshape(8, 128, 8, 128).transpose(2, 1, 0, 3)).astype(bf16),
            "wvT": np.ascontiguousarray(w_qkv[2 * D:3 * D][s].T).astype(bf16),
            "bqk": np.ascontiguousarray(
                np.concatenate([b_qkv[0:D][s], b_qkv[D:2 * D][s]])
                .reshape(8, 128).T).astype(np.float32),
            "bv": b_qkv[2 * D:3 * D][s][None].astype(bf16),
            "woT": np.ascontiguousarray(w_out[:, s].T).astype(bf16),
            "bout": (b_out if g == 0 else np.zeros_like(b_out))[None].astype(bf16),
            "cosT": cosT,
            "sinT": sinT,
        })
    res = run_bass_kernel_spmd(nc, in_maps, list(range(8)))
    _NC_CACHE["last_results"] = res
    parts = [r["out"] for r in res.results]
    return np.stack([parts[2 * b] + parts[2 * b + 1] for b in range(4)]).astype(np.float32)
